# revision 1
# baseline (speedup 1.0000x reference)
"""Multi-head self-attention Trainium2 kernel (8 NeuronCores, SPMD).

Problem: B=2, N=4096, D=512, H=8 heads of dim 64.
  qkv = x @ qkv_w.T + qkv_b ; per-head attention with softmax(QK^T/8) ;
  out = attn @ out_w.T + out_b

Sharding: 16 (batch, head) pairs -> 8 cores, each core owns one batch b and
one head-PAIR (2 adjacent heads = a 128-row slice of the qkv projections).
Each core computes the full attention for its 2 heads over all 4096 rows and
a partial output projection; the host sums the 4 per-batch partials and adds
the (folded) biases.

On-chip layout strategy: everything is computed with the contraction dim on
partitions so no transposes are ever needed:
  Q^T,K^T [128d, 4096]  <- lhsT=W^T tiles, rhs=x^T
  V       [4096, 128d]  (natural; lhsT=x^T tile, rhs=Wv^T) + fused ones column
  S^T = K^T-stationary matmul, 2 heads row-packed (K=64 each) in the PE array
  P^T = exp(S^T) on ScalarE straight out of PSUM (no max-subtraction: |S|<~3)
  O^T accum = (V|1)-stationary matmul over P^T; row 64 = softmax denominator
  normalize via reciprocal + PE outer-product broadcast; partial y^T = Wout^T
  slice-stationary matmul.
Scale 1/sqrt(64) and all biases are folded on the host (wq*=0.125 etc.).
"""

import os
import numpy as np
import ml_dtypes

B, N, D, H, HD = 2, 4096, 512, 8, 64
NCORES = 8
KT_TILES = 4      # D / 128 contraction tiles
JT = 32           # N / 128 key tiles
ICH = 8           # N / 512 query chunks
P = 128

# compute dtype mode: "bf16" (fast), "mixed" (fp32 scores, bf16 PV),
# "fp32" (all fp32)
MODE = os.environ.get("ATTN_KERNEL_MODE", "bf16")

_BUILD_CACHE = {}


def _np_dt(dt):
    import concourse.mybir as mybir
    return np.dtype(ml_dtypes.bfloat16) if dt == mybir.dt.bfloat16 else np.dtype(np.float32)


def _build(mode):
    """Build (and cache) the compiled Bass program for all cores (SPMD)."""
    if mode in _BUILD_CACHE:
        return _BUILD_CACHE[mode]

    import concourse.bacc as bacc
    import concourse.mybir as mybir
    import concourse.tile as tile
    from contextlib import ExitStack

    f32 = mybir.dt.float32
    bf16 = mybir.dt.bfloat16
    if mode == "bf16":
        dt_qk, dt_pv = bf16, bf16
    elif mode == "mixed":
        dt_qk, dt_pv = f32, bf16
    else:
        dt_qk, dt_pv = f32, f32

    Exp = mybir.ActivationFunctionType.Exp

    nc = bacc.Bacc(None, target_bir_lowering=False)
    xt_d = nc.dram_tensor("xt", [KT_TILES, P, N], dt_qk, kind="ExternalInput")
    wqt_d = nc.dram_tensor("wqt", [KT_TILES, P, P], dt_qk, kind="ExternalInput")
    wkt_d = nc.dram_tensor("wkt", [KT_TILES, P, P], dt_qk, kind="ExternalInput")
    wvt_d = nc.dram_tensor("wvt", [KT_TILES, P, P], dt_qk, kind="ExternalInput")
    wot_d = nc.dram_tensor("wot", [2, HD, D], dt_pv, kind="ExternalInput")
    bq_d = nc.dram_tensor("bq", [P, 1], f32, kind="ExternalInput")
    bk_d = nc.dram_tensor("bk", [P, 1], f32, kind="ExternalInput")
    yp_d = nc.dram_tensor("yp", [KT_TILES, P, N], f32, kind="ExternalOutput")

    def ics(i):
        return slice(i * 512, (i + 1) * 512)

    def jts(j):
        return slice(j * P, (j + 1) * P)

    def mts(m):
        return slice(m * P, (m + 1) * P)

    with tile.TileContext(nc) as tc, ExitStack() as ctx:
        const = ctx.enter_context(tc.tile_pool(name="const", bufs=1))
        sp = ctx.enter_context(tc.tile_pool(name="spool", bufs=4, space="PSUM"))
        op = ctx.enter_context(tc.tile_pool(name="opool", bufs=3, space="PSUM"))
        mp = ctx.enter_context(tc.tile_pool(name="mpool", bufs=1, space="PSUM"))
        pp = ctx.enter_context(tc.tile_pool(name="ppool", bufs=6))
        yep = ctx.enter_context(tc.tile_pool(name="yepool", bufs=3))
        rrp = ctx.enter_context(tc.tile_pool(name="rrpool", bufs=2))
        rbp = ctx.enter_context(tc.tile_pool(name="rbpool", bufs=2))

        xt = const.tile([P, KT_TILES, N], dt_qk, tag="xt")
        wqt = const.tile([P, KT_TILES, P], dt_qk, tag="wqt")
        wkt = const.tile([P, KT_TILES, P], dt_qk, tag="wkt")
        wvt = const.tile([P, KT_TILES, P], dt_qk, tag="wvt")
        for k in range(KT_TILES):
            nc.sync.dma_start(xt[:, k, :], xt_d[k])
            nc.sync.dma_start(wqt[:, k, :], wqt_d[k])
            nc.sync.dma_start(wkt[:, k, :], wkt_d[k])
            nc.sync.dma_start(wvt[:, k, :], wvt_d[k])
        wot = const.tile([HD, 2, D], dt_pv, tag="wot")
        for h in range(2):
            nc.sync.dma_start(wot[:, h, :], wot_d[h])
        bq = const.tile([P, 1], f32, tag="bq")
        bk = const.tile([P, 1], f32, tag="bk")
        nc.sync.dma_start(bq[:], bq_d[:])
        nc.sync.dma_start(bk[:], bk_d[:])

        QT = const.tile([P, N], dt_qk, tag="QT")
        KT = const.tile([P, N], dt_qk, tag="KT")
        Vp = const.tile([P, JT, 130], dt_pv, tag="Vp")
        OT0 = const.tile([HD, N], dt_pv, tag="OT0")
        OT1 = const.tile([HD, N], dt_pv, tag="OT1")
        ones = const.tile([1, HD], f32, tag="ones")
        nc.vector.memset(ones[:], 1.0)
        nc.vector.memset(Vp[:, :, 64:65], 1.0)
        nc.vector.memset(Vp[:, :, 129:130], 1.0)

        # ---- Q^T / K^T projections: psum <- sum_k wt[k].T @ xt[k] ----
        for ic in range(ICH):
            for wt, bt, dst in ((wqt, bq, QT), (wkt, bk, KT)):
                ps = sp.tile([P, 512], f32, tag="s")
                for k in range(KT_TILES):
                    nc.tensor.matmul(
                        ps[:], wt[:, k, :], xt[:, k, ics(ic)],
                        start=(k == 0), stop=(k == KT_TILES - 1),
                    )
                nc.vector.tensor_scalar_add(dst[:, ics(ic)], ps[:], bt[:, 0:1])

        # ---- V projection (natural layout): psum <- sum_k xt[k,jt].T @ wvt[k] ----
        for jt in range(JT):
            ps = op.tile([P, P], f32, tag="o")
            for k in range(KT_TILES):
                nc.tensor.matmul(
                    ps[:], xt[:, k, jts(jt)], wvt[:, k, :],
                    start=(k == 0), stop=(k == KT_TILES - 1),
                )
            nc.vector.tensor_copy(Vp[:, jt, 0:64], ps[:, 0:64])
            nc.vector.tensor_copy(Vp[:, jt, 65:129], ps[:, 64:128])

        # ---- attention ----
        for ic in range(ICH):
            o0 = op.tile([65, 512], f32, tag="o")
            o1 = op.tile([65, 512], f32, tag="o")
            for jt in range(JT):
                s0 = sp.tile([P, 512], f32, tag="s")
                s1 = sp.tile([P, 512], f32, tag="s")
                nc.tensor.matmul(s0[:], KT[0:64, jts(jt)], QT[0:64, ics(ic)],
                                 start=True, stop=True, tile_position=(0, 0))
                nc.tensor.matmul(s1[:], KT[64:128, jts(jt)], QT[64:128, ics(ic)],
                                 start=True, stop=True, tile_position=(64, 0))
                p0 = pp.tile([P, 512], dt_pv, tag="p")
                p1 = pp.tile([P, 512], dt_pv, tag="p")
                nc.scalar.activation(p0[:], s0[:], Exp)
                nc.scalar.activation(p1[:], s1[:], Exp)
                nc.tensor.matmul(o0[:], Vp[:, jt, 0:65], p0[:],
                                 start=(jt == 0), stop=(jt == JT - 1))
                nc.tensor.matmul(o1[:], Vp[:, jt, 65:130], p1[:],
                                 start=(jt == 0), stop=(jt == JT - 1))
            # normalize each head: OT[:, ic] = o[0:64] * (1/r) with r = o[64]
            for o, OTt in ((o0, OT0), (o1, OT1)):
                rr = rrp.tile([1, 512], f32, tag="rr")
                nc.vector.reciprocal(rr[:], o[64:65, :])
                rb = mp.tile([HD, 512], f32, tag="mp")
                nc.tensor.matmul(rb[:], ones[:], rr[:], start=True, stop=True)
                rbs = rbp.tile([HD, 512], f32, tag="rbs")
                nc.vector.tensor_copy(rbs[:], rb[:])
                nc.vector.tensor_mul(OTt[:, ics(ic)], o[0:64, :], rbs[:])
            # partial output projection: y^T[mt, ic] = sum_h wot[h,mt].T @ OTh[ic]
            for mt in range(KT_TILES):
                yps = mp.tile([P, 512], f32, tag="mp")
                nc.tensor.matmul(yps[:], wot[:, 0, mts(mt)], OT0[:, ics(ic)],
                                 start=True, stop=False)
                nc.tensor.matmul(yps[:], wot[:, 1, mts(mt)], OT1[:, ics(ic)],
                                 start=False, stop=True)
                ye = yep.tile([P, 512], f32, tag="ye")
                nc.vector.tensor_copy(ye[:], yps[:])
                nc.sync.dma_start(yp_d[mt, :, ics(ic)], ye[:])

    nc.compile()
    _BUILD_CACHE[mode] = nc
    return nc


def _prep_inputs(x, qkv_w, qkv_b, out_w, mode):
    """Per-core input maps. Core c: batch c//4, head-pair c%4."""
    if mode == "bf16":
        dt_qk = np.dtype(ml_dtypes.bfloat16)
        dt_pv = dt_qk
    elif mode == "mixed":
        dt_qk = np.dtype(np.float32)
        dt_pv = np.dtype(ml_dtypes.bfloat16)
    else:
        dt_qk = np.dtype(np.float32)
        dt_pv = dt_qk

    x = np.asarray(x, np.float32)
    qkv_w = np.asarray(qkv_w, np.float32)
    qkv_b = np.asarray(qkv_b, np.float32)
    out_w = np.asarray(out_w, np.float32)

    xts = []
    for b in range(B):
        xt = np.ascontiguousarray(x[b].T).reshape(KT_TILES, P, N)
        xts.append(xt.astype(dt_qk))

    in_maps = []
    for c in range(NCORES):
        b, m = divmod(c, 4)
        rs = slice(P * m, P * (m + 1))
        wq = (0.125 * qkv_w[0:D][rs]).T.reshape(KT_TILES, P, P)
        wk = qkv_w[D:2 * D][rs].T.reshape(KT_TILES, P, P)
        wv = qkv_w[2 * D:3 * D][rs].T.reshape(KT_TILES, P, P)
        wo = np.ascontiguousarray(out_w[:, rs].T).reshape(2, HD, D)
        in_maps.append({
            "xt": xts[b],
            "wqt": np.ascontiguousarray(wq).astype(dt_qk),
            "wkt": np.ascontiguousarray(wk).astype(dt_qk),
            "wvt": np.ascontiguousarray(wv).astype(dt_qk),
            "wot": wo.astype(dt_pv),
            "bq": (0.125 * qkv_b[0:D][rs]).reshape(P, 1).astype(np.float32),
            "bk": qkv_b[D:2 * D][rs].reshape(P, 1).astype(np.float32),
        })
    return in_maps


def _gather(results, qkv_b, out_w, out_b):
    # y[b] = (sum over the batch's 4 cores of yp)^T + out_w @ bv + out_b
    bias_vec = out_w.astype(np.float32) @ np.asarray(qkv_b, np.float32)[2 * D:3 * D] \
        + np.asarray(out_b, np.float32)
    y = np.empty((B, N, D), np.float32)
    for b in range(B):
        acc = np.zeros((D, N), np.float32)
        for m in range(4):
            acc += results[4 * b + m]["yp"].reshape(D, N)
        y[b] = acc.T + bias_vec
    return y


def _run(inputs, trace=False, tmpdir=None):
    from concourse.bass_utils import run_bass_kernel_spmd

    nc = _build(MODE)
    in_maps = _prep_inputs(inputs["x"], inputs["qkv_w"], inputs["qkv_b"],
                           inputs["out_w"], MODE)
    kw = {}
    if trace:
        kw = dict(trace=True, tmpdir=tmpdir)
    res = run_bass_kernel_spmd(nc, in_maps, core_ids=list(range(NCORES)), **kw)
    y = _gather(res.results, inputs["qkv_b"], inputs["out_w"], inputs["out_b"])
    return y, res


def kernel(x, qkv_w, qkv_b, out_w, out_b):
    y, _ = _run(dict(x=x, qkv_w=qkv_w, qkv_b=qkv_b, out_w=out_w, out_b=out_b))
    return y


# revision 5
# speedup vs baseline: 1.4215x; 1.4215x over previous
"""Multi-head self-attention Trainium2 kernel (8 NeuronCores, SPMD).

Problem: B=2, N=4096, D=512, H=8 heads of dim 64.
  qkv = x @ qkv_w.T + qkv_b ; per-head attention with softmax(QK^T/8) ;
  out = attn @ out_w.T + out_b

Sharding: 16 (batch, head) pairs -> 8 cores, each core owns one batch b and
one head-PAIR (2 adjacent heads = a 128-row slice of the qkv projections).
Each core computes the full attention for its 2 heads over all 4096 rows and
a partial output projection; the host sums the 4 per-batch partials and adds
the (folded) biases.

On-chip layout strategy: everything is computed with the contraction dim on
partitions so no transposes are ever needed:
  Q^T,K^T [128d, 4096]  <- lhsT=W^T tiles, rhs=x^T
  V       [4096, 128d]  (natural; lhsT=x^T tile, rhs=Wv^T) + fused ones column
  S^T = K^T-stationary matmul, 2 heads row-packed (K=64 each) in the PE array
  P^T = exp(S^T) on ScalarE straight out of PSUM (no max-subtraction: |S|<~3)
  O^T accum = (V|1)-stationary matmul over P^T; row 64 = softmax denominator
  normalize via reciprocal + PE outer-product broadcast; partial y^T = Wout^T
  slice-stationary matmul.
Scale 1/sqrt(64) and all biases are folded on the host (wq*=0.125 etc.).
"""

import os
import numpy as np
import ml_dtypes

B, N, D, H, HD = 2, 4096, 512, 8, 64
NCORES = 8
KT_TILES = 4      # D / 128 contraction tiles
JT = 32           # N / 128 key tiles
ICH = 8           # N / 512 query chunks
P = 128

# compute dtype mode: "bf16" (fast), "mixed" (fp32 scores, bf16 PV),
# "fp32" (all fp32)
MODE = os.environ.get("ATTN_KERNEL_MODE", "bf16")

_BUILD_CACHE = {}


def _np_dt(dt):
    import concourse.mybir as mybir
    return np.dtype(ml_dtypes.bfloat16) if dt == mybir.dt.bfloat16 else np.dtype(np.float32)


def _build(mode):
    """Build (and cache) the compiled Bass program for all cores (SPMD)."""
    if mode in _BUILD_CACHE:
        return _BUILD_CACHE[mode]

    import concourse.bacc as bacc
    import concourse.mybir as mybir
    import concourse.tile as tile
    from contextlib import ExitStack

    f32 = mybir.dt.float32
    bf16 = mybir.dt.bfloat16
    if mode == "bf16":
        dt_qk, dt_pv = bf16, bf16
    elif mode == "mixed":
        dt_qk, dt_pv = f32, bf16
    else:
        dt_qk, dt_pv = f32, f32

    Exp = mybir.ActivationFunctionType.Exp

    nc = bacc.Bacc(None, target_bir_lowering=False)
    xt_d = nc.dram_tensor("xt", [KT_TILES, P, N], dt_qk, kind="ExternalInput")
    wqt_d = nc.dram_tensor("wqt", [KT_TILES, P, P], dt_qk, kind="ExternalInput")
    wkt_d = nc.dram_tensor("wkt", [KT_TILES, P, P], dt_qk, kind="ExternalInput")
    wvt_d = nc.dram_tensor("wvt", [KT_TILES, P, P], dt_qk, kind="ExternalInput")
    wot_d = nc.dram_tensor("wot", [2, HD, D], dt_pv, kind="ExternalInput")
    bq_d = nc.dram_tensor("bq", [P, 1], f32, kind="ExternalInput")
    bk_d = nc.dram_tensor("bk", [P, 1], f32, kind="ExternalInput")
    yp_d = nc.dram_tensor("yp", [KT_TILES, P, N], f32, kind="ExternalOutput")

    def ics(i):
        return slice(i * 512, (i + 1) * 512)

    def jts(j):
        return slice(j * P, (j + 1) * P)

    def mts(m):
        return slice(m * P, (m + 1) * P)

    with tile.TileContext(nc) as tc, ExitStack() as ctx:
        const = ctx.enter_context(tc.tile_pool(name="const", bufs=1))
        sp = ctx.enter_context(tc.tile_pool(name="spool", bufs=2, space="PSUM"))
        op = ctx.enter_context(tc.tile_pool(name="opool", bufs=3, space="PSUM"))
        mp = ctx.enter_context(tc.tile_pool(name="mpool", bufs=1, space="PSUM"))
        pp = ctx.enter_context(tc.tile_pool(name="ppool", bufs=6))
        yep = ctx.enter_context(tc.tile_pool(name="yepool", bufs=3))
        rrp = ctx.enter_context(tc.tile_pool(name="rrpool", bufs=2))
        rbp = ctx.enter_context(tc.tile_pool(name="rbpool", bufs=2))

        xt = const.tile([P, KT_TILES, N], dt_qk, tag="xt")
        wqt = const.tile([P, KT_TILES, P], dt_qk, tag="wqt")
        wkt = const.tile([P, KT_TILES, P], dt_qk, tag="wkt")
        wvt = const.tile([P, KT_TILES, P], dt_qk, tag="wvt")
        for k in range(KT_TILES):
            nc.sync.dma_start(xt[:, k, :], xt_d[k])
            nc.sync.dma_start(wqt[:, k, :], wqt_d[k])
            nc.sync.dma_start(wkt[:, k, :], wkt_d[k])
            nc.sync.dma_start(wvt[:, k, :], wvt_d[k])
        wot = const.tile([HD, 2, D], dt_pv, tag="wot")
        for h in range(2):
            nc.sync.dma_start(wot[:, h, :], wot_d[h])
        bq = const.tile([P, 1], f32, tag="bq")
        bk = const.tile([P, 1], f32, tag="bk")
        nc.sync.dma_start(bq[:], bq_d[:])
        nc.sync.dma_start(bk[:], bk_d[:])

        QT = const.tile([P, N], dt_qk, tag="QT")
        KT = const.tile([P, N], dt_qk, tag="KT")
        Vp = const.tile([P, JT, 130], dt_pv, tag="Vp")
        OT0 = const.tile([HD, N], dt_pv, tag="OT0")
        OT1 = const.tile([HD, N], dt_pv, tag="OT1")
        ones = const.tile([1, HD], f32, tag="ones")
        nc.vector.memset(ones[:], 1.0)
        nc.vector.memset(Vp[:, :, 64:65], 1.0)
        nc.vector.memset(Vp[:, :, 129:130], 1.0)

        # ---- Q^T / K^T projections: psum <- sum_k wt[k].T @ xt[k] ----
        for ic in range(ICH):
            for wt, bt, dst in ((wqt, bq, QT), (wkt, bk, KT)):
                ps = sp.tile([P, 512], f32, tag="s")
                for k in range(KT_TILES):
                    nc.tensor.matmul(
                        ps[:], wt[:, k, :], xt[:, k, ics(ic)],
                        start=(k == 0), stop=(k == KT_TILES - 1),
                    )
                nc.vector.tensor_scalar_add(dst[:, ics(ic)], ps[:], bt[:, 0:1])

        # ---- V projection (natural layout): psum <- sum_k xt[k,jt].T @ wvt[k] ----
        for jt in range(JT):
            ps = op.tile([P, P], f32, tag="o")
            for k in range(KT_TILES):
                nc.tensor.matmul(
                    ps[:], xt[:, k, jts(jt)], wvt[:, k, :],
                    start=(k == 0), stop=(k == KT_TILES - 1),
                )
            nc.vector.tensor_copy(Vp[:, jt, 0:64], ps[:, 0:64])
            nc.vector.tensor_copy(Vp[:, jt, 65:129], ps[:, 64:128])

        # ---- attention (software-pipelined emission: S/exp of step t, PV of
        # step t-1, so the PE never queues a PV behind the exp it feeds) ----
        def finalize(ic, o0, o1):
            # normalize each head: OT[:, ic] = o[0:64] * (1/r) with r = o[64]
            for o, OTt in ((o0, OT0), (o1, OT1)):
                rr = rrp.tile([1, 512], f32, tag="rr")
                nc.vector.reciprocal(rr[:], o[64:65, :])
                rb = mp.tile([HD, 512], f32, tag="mp")
                nc.tensor.matmul(rb[:], ones[:], rr[:], start=True, stop=True)
                rbs = rbp.tile([HD, 512], f32, tag="rbs")
                nc.vector.tensor_copy(rbs[:], rb[:])
                nc.vector.tensor_mul(OTt[:, ics(ic)], o[0:64, :], rbs[:])
            # partial output projection: y^T[mt, ic] = sum_h wot[h,mt].T @ OTh[ic]
            for mt in range(KT_TILES):
                yps = mp.tile([P, 512], f32, tag="mp")
                nc.tensor.matmul(yps[:], wot[:, 0, mts(mt)], OT0[:, ics(ic)],
                                 start=True, stop=False)
                nc.tensor.matmul(yps[:], wot[:, 1, mts(mt)], OT1[:, ics(ic)],
                                 start=False, stop=True)
                ye = yep.tile([P, 512], f32, tag="ye")
                nc.vector.tensor_copy(ye[:], yps[:])
                nc.sync.dma_start(yp_d[mt, :, ics(ic)], ye[:])

        otiles = {}
        pend = None  # (p_tile, ic, jt) whose PV is not yet emitted
        for ic in range(ICH):
            otiles[ic] = (op.tile([65, 512], f32, tag="o", name=f"o0_{ic}"),
                          op.tile([65, 512], f32, tag="o", name=f"o1_{ic}"))
            for jt in range(JT):
                s = sp.tile([P, 1024], f32, tag="s")
                nc.tensor.matmul(s[:, 0:512], KT[0:64, jts(jt)],
                                 QT[0:64, ics(ic)],
                                 start=True, stop=True, tile_position=(0, 0))
                nc.tensor.matmul(s[:, 512:1024], KT[64:128, jts(jt)],
                                 QT[64:128, ics(ic)],
                                 start=True, stop=True, tile_position=(64, 0))
                p = pp.tile([P, 1024], dt_pv, tag="p")
                nc.scalar.activation(p[:], s[:], Exp)
                if pend is not None:
                    pp_, pic, pjt = pend
                    o0, o1 = otiles[pic]
                    nc.tensor.matmul(o0[:], Vp[:, pjt, 0:65], pp_[:, 0:512],
                                     start=(pjt == 0), stop=(pjt == JT - 1))
                    nc.tensor.matmul(o1[:], Vp[:, pjt, 65:130], pp_[:, 512:1024],
                                     start=(pjt == 0), stop=(pjt == JT - 1))
                    if pjt == JT - 1:
                        finalize(pic, o0, o1)
                pend = (p, ic, jt)
        pp_, pic, pjt = pend
        o0, o1 = otiles[pic]
        nc.tensor.matmul(o0[:], Vp[:, pjt, 0:65], pp_[:, 0:512],
                         start=(pjt == 0), stop=(pjt == JT - 1))
        nc.tensor.matmul(o1[:], Vp[:, pjt, 65:130], pp_[:, 512:1024],
                         start=(pjt == 0), stop=(pjt == JT - 1))
        finalize(pic, o0, o1)

    nc.compile()
    _BUILD_CACHE[mode] = nc
    return nc


def _prep_inputs(x, qkv_w, qkv_b, out_w, mode):
    """Per-core input maps. Core c: batch c//4, head-pair c%4."""
    if mode == "bf16":
        dt_qk = np.dtype(ml_dtypes.bfloat16)
        dt_pv = dt_qk
    elif mode == "mixed":
        dt_qk = np.dtype(np.float32)
        dt_pv = np.dtype(ml_dtypes.bfloat16)
    else:
        dt_qk = np.dtype(np.float32)
        dt_pv = dt_qk

    x = np.asarray(x, np.float32)
    qkv_w = np.asarray(qkv_w, np.float32)
    qkv_b = np.asarray(qkv_b, np.float32)
    out_w = np.asarray(out_w, np.float32)

    xts = []
    for b in range(B):
        xt = np.ascontiguousarray(x[b].T).reshape(KT_TILES, P, N)
        xts.append(xt.astype(dt_qk))

    in_maps = []
    for c in range(NCORES):
        b, m = divmod(c, 4)
        rs = slice(P * m, P * (m + 1))
        wq = (0.125 * qkv_w[0:D][rs]).T.reshape(KT_TILES, P, P)
        wk = qkv_w[D:2 * D][rs].T.reshape(KT_TILES, P, P)
        wv = qkv_w[2 * D:3 * D][rs].T.reshape(KT_TILES, P, P)
        wo = np.ascontiguousarray(out_w[:, rs].T).reshape(2, HD, D)
        in_maps.append({
            "xt": xts[b],
            "wqt": np.ascontiguousarray(wq).astype(dt_qk),
            "wkt": np.ascontiguousarray(wk).astype(dt_qk),
            "wvt": np.ascontiguousarray(wv).astype(dt_qk),
            "wot": wo.astype(dt_pv),
            "bq": (0.125 * qkv_b[0:D][rs]).reshape(P, 1).astype(np.float32),
            "bk": qkv_b[D:2 * D][rs].reshape(P, 1).astype(np.float32),
        })
    return in_maps


def _gather(results, qkv_b, out_w, out_b):
    # y[b] = (sum over the batch's 4 cores of yp)^T + out_w @ bv + out_b
    bias_vec = out_w.astype(np.float32) @ np.asarray(qkv_b, np.float32)[2 * D:3 * D] \
        + np.asarray(out_b, np.float32)
    y = np.empty((B, N, D), np.float32)
    for b in range(B):
        acc = np.zeros((D, N), np.float32)
        for m in range(4):
            acc += results[4 * b + m]["yp"].reshape(D, N)
        y[b] = acc.T + bias_vec
    return y


def _run(inputs, trace=False, tmpdir=None):
    from concourse.bass_utils import run_bass_kernel_spmd

    nc = _build(MODE)
    in_maps = _prep_inputs(inputs["x"], inputs["qkv_w"], inputs["qkv_b"],
                           inputs["out_w"], MODE)
    kw = {}
    if trace:
        kw = dict(trace=True, tmpdir=tmpdir)
    res = run_bass_kernel_spmd(nc, in_maps, core_ids=list(range(NCORES)), **kw)
    y = _gather(res.results, inputs["qkv_b"], inputs["out_w"], inputs["out_b"])
    return y, res


def kernel(x, qkv_w, qkv_b, out_w, out_b):
    y, _ = _run(dict(x=x, qkv_w=qkv_w, qkv_b=qkv_b, out_w=out_w, out_b=out_b))
    return y
